# revision 4
# baseline (speedup 1.0000x reference)
"""Trainium2 Bass kernel for Ernie4.5-VL attention (mRoPE + GQA causal attention).

Sharding: tensor-parallel over heads across 8 cores. Each core computes
2 q heads + its kv head (replicated per core pair): qkv projection
(q/k feature-major, V token-major directly — no transposes), interleaved
mRoPE (via a host-side even/odd column permutation of the q/k weight
slices so the rotation becomes two contiguous partition halves), causal
attention with unnormalized softmax (denominator via bf16 tile adds +
one all-ones matmul), and the o_proj partial product. Host sums the 8
partial outputs.

All tensors move through SBUF/DRAM as bf16; matmuls are bf16 in / fp32
psum out; psum evacuations round once to bf16. Schedule: a flat
software pipeline where attention chunk g's score->exp->AV steps are
interleaved (emission-order round-robin) with chunk g+1's projection
matmuls and chunk g-1's o_proj — the PE fills exp (ACT) latency with
projection work instead of stalling, which also keeps the PE p-state
ramped.
"""
import numpy as np
import ml_dtypes
from contextlib import ExitStack

import concourse.bacc as bacc
import concourse.tile as tile
from concourse import mybir
from concourse.bass_utils import run_bass_kernel_spmd

HIDDEN = 2048
T = 2048
N_HEADS = 16
N_KV = 4
HD = 128
THETA = 500000.0
NCORES = 8
SCALE = HD ** -0.5

F32 = mybir.dt.float32
BF16 = mybir.dt.bfloat16
I32 = mybir.dt.int32

# within-head column permutation: evens then odds (so interleaved rope pairs
# become two contiguous partition halves in feature-major layout)
PERM = np.concatenate([np.arange(0, HD, 2), np.arange(1, HD, 2)])
# pair index p (0..63): p<44: even->pos row 1 (h), odd->row 2 (w); p>=44: row 0 (t)
ROW_MAP = np.array([(1 if p % 2 == 0 else 2) if p < 44 else 0 for p in range(64)])
INVF = (THETA ** (-(np.arange(64, dtype=np.float64) / 64))).astype(np.float32)

NT = T // 128      # 16 token tiles
NG = T // 512      # 4 token chunks
NH_T = HIDDEN // 128  # 16 hidden tiles


def _build(dbg=False):
    nc = bacc.Bacc("TRN2", target_bir_lowering=False, debug=False)
    d_xT = nc.dram_tensor("xT", [HIDDEN, T], BF16, kind="ExternalInput").ap()
    d_w = nc.dram_tensor("w_slice", [HIDDEN, 512], BF16, kind="ExternalInput").ap()
    d_wo = nc.dram_tensor("wo_slice", [256, HIDDEN], BF16, kind="ExternalInput").ap()
    d_pos = nc.dram_tensor("pos_sel", [128, T], I32, kind="ExternalInput").ap()
    d_invf = nc.dram_tensor("invf", [128, 1], F32, kind="ExternalInput").ap()
    d_svec = nc.dram_tensor("svec", [128, 1], F32, kind="ExternalInput").ap()
    d_mL = nc.dram_tensor("mask_l", [128, 128], BF16, kind="ExternalInput").ap()
    d_mR = nc.dram_tensor("mask_r", [128, 4, 512], BF16, kind="ExternalInput").ap()
    d_ones = nc.dram_tensor("ones", [128, 128], BF16, kind="ExternalInput").ap()
    d_yT = nc.dram_tensor("yT", [HIDDEN, T], BF16, kind="ExternalOutput").ap()
    if dbg:
        d_qkv = nc.dram_tensor("dbg_qkv", [128, 3, T], F32, kind="ExternalOutput").ap()
        d_cs = nc.dram_tensor("dbg_cs", [128, 2, T], F32, kind="ExternalOutput").ap()
        d_V = nc.dram_tensor("dbg_V", [128, NT, 128], F32, kind="ExternalOutput").ap()
        d_O = nc.dram_tensor("dbg_O", [128, 2, T], F32, kind="ExternalOutput").ap()

    TWO_PI = float(2 * np.pi)
    with tile.TileContext(nc) as tc, ExitStack() as ctx:
        const = ctx.enter_context(tc.tile_pool(name="const", bufs=1))
        big = ctx.enter_context(tc.tile_pool(name="big", bufs=1))

        # resident tiles
        w_sb = const.tile([128, NH_T, 512], BF16)       # qkv weight slice
        wo_sb = const.tile([128, 2, HIDDEN], BF16)      # o_proj rows
        mL_sb = const.tile([128, 128], BF16)            # causal mask, left factor
        mR_sb = const.tile([128, 4, 512], BF16)         # causal mask, right factor
        ones_sb = const.tile([128, 128], BF16)
        invf_sb = const.tile([128, 1], F32)
        svec_sb = const.tile([128, 1], F32)
        pos_sb = const.tile([128, T], I32)
        qkv_sb = big.tile([128, 3, T], BF16)            # q0|q1|k feature-major (roped)
        V_sb = big.tile([128, NT, 128], BF16)           # V token-major
        O_sb = big.tile([128, 2, T], BF16)              # attention out, feature-major
        cdup = big.tile([128, T], BF16)                 # cos table (dup halves)
        sflip = big.tile([128, T], BF16)                # sin table ([-s; s])

        # PSUM budget (8 banks): q0/q1/k accum 3 + V-direct 1 +
        # shared(scores/o_proj) 3 + AV accum 1.
        xtp = ctx.enter_context(tc.tile_pool(name="xt", bufs=2))
        qkvp = ctx.enter_context(tc.tile_pool(name="qkvp", bufs=3, space="PSUM"))
        vdp = ctx.enter_context(tc.tile_pool(name="vdp", bufs=1, space="PSUM"))
        spp = ctx.enter_context(tc.tile_pool(name="spp", bufs=3, space="PSUM"))
        avp = ctx.enter_context(tc.tile_pool(name="avp", bufs=1, space="PSUM"))
        tbl = ctx.enter_context(tc.tile_pool(name="tbl", bufs=1))
        rp = ctx.enter_context(tc.tile_pool(name="rope", bufs=2))
        ep = ctx.enter_context(tc.tile_pool(name="ep", bufs=6))
        rv = ctx.enter_context(tc.tile_pool(name="rv", bufs=2))
        racc = ctx.enter_context(tc.tile_pool(name="racc", bufs=2))
        yo = ctx.enter_context(tc.tile_pool(name="yo", bufs=2))

        # ---- startup DMAs, ordered so the first projection matmuls and the
        # chunk-0 rope tables unblock as early as possible
        xt_tiles = {}
        xt_tiles[0] = xtp.tile([128, NH_T, 512], BF16, tag="xt", name="xt_0")
        for q4 in range(4):
            nc.sync.dma_start(
                out=w_sb[:, 4 * q4:4 * (q4 + 1), :],
                in_=d_w[512 * q4:512 * (q4 + 1), :].rearrange(
                    "(a p) c -> p a c", p=128))
            nc.sync.dma_start(
                out=xt_tiles[0][:, 4 * q4:4 * (q4 + 1), :],
                in_=d_xT[512 * q4:512 * (q4 + 1), 0:512].rearrange(
                    "(a p) c -> p a c", p=128))
            if q4 == 0:
                nc.sync.dma_start(out=pos_sb[:, 0:512], in_=d_pos[:, 0:512])
                nc.sync.dma_start(out=invf_sb, in_=d_invf)
                nc.sync.dma_start(out=svec_sb, in_=d_svec)
        nc.sync.dma_start(out=pos_sb[:, 512:T], in_=d_pos[:, 512:T])
        nc.sync.dma_start(out=mL_sb, in_=d_mL)
        nc.sync.dma_start(out=mR_sb, in_=d_mR)
        nc.sync.dma_start(out=ones_sb, in_=d_ones)
        nc.sync.dma_start(
            out=wo_sb, in_=d_wo.rearrange("(a p) c -> p a c", p=128))

        # ---- rope tables (all chunks upfront: keeps Sin/Exp from thrashing
        # the ACT function table). invf is pre-divided by 2pi on host:
        # turns = pos * invf; reduce to [-0.5, 0.5] turns via rne f32->i32
        # roundtrip; Sin with 2pi (and per-half sign) folded into ACT scale.
        #   cdup = cos(ang) both halves; sflip = [-sin; +sin] (svec = +-2pi)
        for gt in range(NG):
            tsl2 = np.s_[512 * gt:512 * (gt + 1)]
            turns = tbl.tile([128, 512], F32, tag="turns", name=f"turns{gt}")
            turns_c = tbl.tile([128, 512], F32, tag="turnsc", name=f"turnsc{gt}")
            tint = tbl.tile([128, 512], I32, tag="ti", name=f"ti{gt}")
            tflt = tbl.tile([128, 512], F32, tag="tf", name=f"tf{gt}")
            nc.gpsimd.tensor_copy(turns[:], pos_sb[:, tsl2])   # int32 -> fp32
            nc.vector.tensor_scalar_mul(turns[:], turns[:], invf_sb[:, 0:1])
            nc.vector.tensor_scalar_add(turns_c[:], turns[:], 0.25)
            nc.gpsimd.tensor_copy(tint[:], turns[:])       # round to nearest
            nc.gpsimd.tensor_copy(tflt[:], tint[:])
            nc.vector.tensor_sub(turns[:], turns[:], tflt[:])
            nc.scalar.activation(sflip[:, tsl2], turns[:],
                                 mybir.ActivationFunctionType.Sin,
                                 bias=0.0, scale=svec_sb[:, 0:1])
            # cos path: +0.25 turns offset (cos x = sin(x + pi/2))
            nc.gpsimd.tensor_copy(tint[:], turns_c[:])
            nc.gpsimd.tensor_copy(tflt[:], tint[:])
            nc.vector.tensor_sub(turns_c[:], turns_c[:], tflt[:])
            nc.scalar.activation(cdup[:, tsl2], turns_c[:],
                                 mybir.ActivationFunctionType.Sin,
                                 bias=0.0, scale=TWO_PI)

        # ================= stage emitters =================
        proj_state = {}

        def proj_units(g):
            """Per-hb units of chunk g's projection. First unit issues the
            xt DMA for g (g=0's was issued at startup)."""
            tsl = np.s_[512 * g:512 * (g + 1)]
            units = []

            def alloc():
                if g not in xt_tiles:
                    xt_tiles[g] = xtp.tile([128, NH_T, 512], BF16, tag="xt",
                                           name=f"xt_{g}")
                    nc.sync.dma_start(
                        out=xt_tiles[g],
                        in_=d_xT[:, tsl].rearrange("(a p) c -> p a c", p=128))
                proj_state[g] = {
                    "ps": [qkvp.tile([128, 512], F32, tag="qkvps",
                                     name=f"qkvps_{g}_{i}") for i in range(3)],
                    "vd": vdp.tile([128, 4, 128], F32, tag="vd", name=f"vd_{g}"),
                }

            def mk(hb):
                def emit():
                    if hb == 0:
                        alloc()
                    st = proj_state[g]
                    xt_b = xt_tiles[g]
                    for i in range(3):
                        nc.tensor.matmul(
                            st["ps"][i][:], w_sb[:, hb, 128 * i:128 * (i + 1)],
                            xt_b[:, hb, :],
                            start=(hb == 0), stop=(hb == NH_T - 1))
                    for tt in range(4):
                        nc.tensor.matmul(
                            st["vd"][:, tt, :],
                            xt_b[:, hb, 128 * tt:128 * (tt + 1)],
                            w_sb[:, hb, 384:512],
                            start=(hb == 0), stop=(hb == NH_T - 1))
                return emit

            for hb in range(NH_T):
                units.append(mk(hb))
            return units

        def evac_rope(g):
            """V evac (Pool) + mRoPE for q0/q1/k of chunk g, reading the
            projection psums directly (swapped halves via psum->SBUF DMA),
            writing bf16 qkv_sb once. k first: it gates chunk g's scores."""
            tsl = np.s_[512 * g:512 * (g + 1)]
            st = proj_state[g]
            xs = rp.tile([128, 3, 512], BF16, tag="xs", name=f"xs{g}")

            def rope_one(t3):
                psx = st["ps"][t3]
                x = qkv_sb[:, t3, tsl]
                xraw = rp.tile([128, 512], BF16, tag="xraw", name=f"xr_{g}_{t3}")
                nc.vector.tensor_copy(xraw[:], psx[:])
                nc.sync.dma_start(out=xs[0:64, t3, :], in_=xraw[64:128, :])
                nc.sync.dma_start(out=xs[64:128, t3, :], in_=xraw[0:64, :])
                t1 = rp.tile([128, 512], F32, tag="t1", name=f"t1_{g}_{t3}")
                t2 = rp.tile([128, 512], F32, tag="t2", name=f"t2_{g}_{t3}")
                nc.vector.tensor_mul(t1[:], psx[:], cdup[:, tsl])
                nc.gpsimd.tensor_mul(t2[:], xs[:, t3, :], sflip[:, tsl])
                nc.vector.tensor_add(x, t1[:], t2[:])

            rope_one(2)
            for tt in range(4):
                nc.gpsimd.tensor_copy(V_sb[:, 4 * g + tt, :], st["vd"][:, tt, :])
            rope_one(0)
            rope_one(1)

        def attn_steps(g):
            """Flat list of per-j-step emitters for both heads of chunk g.
            Scores run one step ahead of AV; head-0's denominator tail is
            emitted two steps into head 1 so its latency hides behind
            head-1 scores."""
            tsl = np.s_[512 * g:512 * (g + 1)]
            jmax = 4 * g + 4
            state = {}

            def head_alloc(h):
                state[h] = {
                    "po": avp.tile([128, 512], F32, tag="av", name=f"po{g}_{h}"),
                    "ra": racc.tile([128, 512], BF16, tag="ra", name=f"ra{g}_{h}"),
                    "rb": racc.tile([128, 512], BF16, tag="rb", name=f"rb{g}_{h}"),
                    "Es": [None] * jmax,
                }

            def mk_step(h, j):
                def emit():
                    if j == 0:
                        head_alloc(h)
                    st = state[h]
                    qc = qkv_sb[:, h, tsl]
                    m = j - 4 * g
                    ps = spp.tile([128, 512], F32, tag="sp", name=f"s{g}_{h}_{j}")
                    nc.tensor.matmul(ps[:], qkv_sb[:, 2, 128 * j:128 * (j + 1)],
                                     qc, start=True, stop=(m < 0))
                    if m >= 0:
                        # additive causal mask (-1e9 on invalid) via rank-
                        # factored matmul accumulated into the scores psum
                        nc.tensor.matmul(ps[:], mL_sb[:], mR_sb[:, m, :],
                                         start=False, stop=True)
                    E = ep.tile([128, 512], BF16, tag="e", name=f"e{g}_{h}_{j}")
                    st["Es"][j] = E
                    nc.scalar.activation(E[:], ps[:],
                                         mybir.ActivationFunctionType.Exp,
                                         scale=SCALE)
                    # row-sum partials: two bf16 accumulators on DVE
                    if j == 0:
                        nc.vector.tensor_copy(st["ra"][:], E[:])
                    elif j == 1:
                        nc.vector.tensor_copy(st["rb"][:], E[:])
                    elif j % 2 == 0:
                        nc.vector.tensor_add(st["ra"][:], st["ra"][:], E[:])
                    else:
                        nc.vector.tensor_add(st["rb"][:], st["rb"][:], E[:])
                    if j >= 1:
                        nc.tensor.matmul(st["po"][:], V_sb[:, j - 1, :],
                                         st["Es"][j - 1][:],
                                         start=(j == 1), stop=False)
                return emit

            def mk_tail(h):
                def emit():
                    st = state[h]
                    nc.tensor.matmul(st["po"][:], V_sb[:, jmax - 1, :],
                                     st["Es"][jmax - 1][:],
                                     start=(jmax == 1), stop=True)
                    nc.vector.tensor_add(st["ra"][:], st["ra"][:], st["rb"][:])
                    # r broadcast across partitions via one all-ones matmul
                    pr = spp.tile([128, 512], F32, tag="sp", name=f"pr{g}_{h}")
                    nc.tensor.matmul(pr[:], ones_sb[:], st["ra"][:],
                                     start=True, stop=True)
                    rinv = rv.tile([128, 512], F32, tag="rv", name=f"rinv{g}_{h}")
                    nc.vector.reciprocal(rinv[:], pr[:])
                    nc.vector.tensor_mul(O_sb[:, h, tsl], st["po"][:], rinv[:])
                return emit

            steps = [mk_step(0, j) for j in range(jmax)]
            h1 = [mk_step(1, j) for j in range(jmax)]
            steps += h1[:2] + [mk_tail(0)] + h1[2:] + [mk_tail(1)]
            return steps

        def oproj_units(g):
            """o_proj partial chunk: yT[:, tsl] = sum_h wo_h.T @ O_h, with
            psum evacuation rotated over DVE/ACT/Pool and a DMA per 4 tiles."""
            tsl = np.s_[512 * g:512 * (g + 1)]
            ybuf = yo.tile([128, NH_T, 512], BF16, tag="yo", name=f"yb{g}")

            def mk(i):
                def emit():
                    py = spp.tile([128, 512], F32, tag="sp", name=f"y{g}_{i}")
                    for h in range(2):
                        nc.tensor.matmul(py[:], wo_sb[:, h, 128 * i:128 * (i + 1)],
                                         O_sb[:, h, tsl],
                                         start=(h == 0), stop=(h == 1))
                    r = i % 4
                    if r == 1:
                        nc.scalar.copy(ybuf[:, i, :], py[:])
                    elif r == 3:
                        nc.gpsimd.tensor_copy(ybuf[:, i, :], py[:])
                    else:
                        nc.vector.tensor_copy(ybuf[:, i, :], py[:])
                    if i % 4 == 3:
                        nc.sync.dma_start(
                            out=d_yT[512 * (i // 4):512 * (i // 4 + 1),
                                     tsl].rearrange("(a p) c -> p a c", p=128),
                            in_=ybuf[:, i - 3:i + 1, :])
                return emit

            return [mk(i) for i in range(NH_T)]

        def interleave(steps, fillers):
            """Emit steps with fillers round-robined in proportionally."""
            done = 0
            for si, s in enumerate(steps):
                s()
                want = (si + 1) * len(fillers) // len(steps)
                while done < want:
                    fillers[done]()
                    done += 1
            while done < len(fillers):
                fillers[done]()
                done += 1

        # ================= schedule =================
        for u in proj_units(0):
            u()
        evac_rope(0)
        for g in range(NG):
            fillers = []
            if g > 0:
                fillers += oproj_units(g - 1)
            if g + 1 < NG:
                pu = proj_units(g + 1)
                # alternate oproj and proj units so psum pressure spreads
                mixed = []
                a, b = fillers, pu
                for i in range(max(len(a), len(b))):
                    if i < len(b):
                        mixed.append(b[i])
                    if i < len(a):
                        mixed.append(a[i])
                fillers = mixed
            interleave(attn_steps(g), fillers)
            if g + 1 < NG:
                evac_rope(g + 1)
        for u in oproj_units(NG - 1):
            u()

        if dbg:
            nc.sync.dma_start(out=d_qkv, in_=qkv_sb[:].bitcast(F32))
            nc.sync.dma_start(out=d_cs[:, 0, :], in_=cdup[:].bitcast(F32))
            nc.sync.dma_start(out=d_cs[:, 1, :], in_=sflip[:].bitcast(F32))
            nc.sync.dma_start(out=d_V, in_=V_sb[:].bitcast(F32))
            nc.sync.dma_start(out=d_O, in_=O_sb[:].bitcast(F32))

    nc.compile()
    return nc


_NC_CACHE = None


def _get_nc():
    global _NC_CACHE
    if _NC_CACHE is None:
        _NC_CACHE = _build()
    return _NC_CACHE


def _host_prep(positions, hidden_states, w_qkv, w_o):
    positions = np.asarray(positions, dtype=np.int32)
    hidden_states = np.asarray(hidden_states, dtype=np.float32)
    w_qkv = np.asarray(w_qkv, dtype=np.float32)
    w_o = np.asarray(w_o, dtype=np.float32)

    bf = ml_dtypes.bfloat16
    xT = np.ascontiguousarray(hidden_states.T).astype(bf)
    pos_sel = np.ascontiguousarray(positions[np.concatenate([ROW_MAP, ROW_MAP])])
    invf = np.ascontiguousarray(
        (np.concatenate([INVF, INVF]) / (2 * np.pi)).astype(np.float32).reshape(128, 1))
    tp = np.float32(2 * np.pi)
    svec = np.concatenate([-tp * np.ones(64, np.float32),
                           tp * np.ones(64, np.float32)]).reshape(128, 1)
    # additive causal mask factors: invalid(dk, dq) = [dq - 128m + 1 <= dk]
    #   = sum_p L[p, dk] * Rm[p, dq],  L[p, dk] = [p <= dk],
    #   Rm[p, dq] = [p == max(dq - 128m + 1, 0)]  (scaled by -1e9)
    mask_l = (np.arange(128)[:, None] <= np.arange(128)[None, :]).astype(np.float32)
    mask_r = np.zeros((128, 4, 512), dtype=np.float32)
    for m in range(4):
        c = np.maximum(np.arange(512) - 128 * m + 1, 0)
        valid_rows = c <= 127
        mask_r[c[valid_rows], m, np.arange(512)[valid_rows]] = -1e9
    # sanity: factored mask == boolean causal mask
    dq = np.arange(512)[None, :]
    dk = np.arange(128)[:, None]
    for m in range(4):
        got = mask_l.T @ mask_r[:, m, :]
        want = np.where(dq < dk + 128 * m, -1e9, 0.0)
        assert np.array_equal(got, want), f"mask factorization wrong for m={m}"
    ones = np.ones((128, 128), dtype=np.float32)

    q_size = N_HEADS * HD
    kv_size = N_KV * HD
    in_maps = []
    for c in range(NCORES):
        cols = [w_qkv[:, 2 * c * HD + PERM], w_qkv[:, (2 * c + 1) * HD + PERM]]
        kc = c // 2
        cols.append(w_qkv[:, q_size + kc * HD + PERM])
        cols.append(w_qkv[:, q_size + kv_size + kc * HD:q_size + kv_size + (kc + 1) * HD])
        w_slice = np.ascontiguousarray(np.concatenate(cols, axis=1)).astype(bf)
        wo_slice = np.ascontiguousarray(w_o[2 * c * HD:(2 * c + 2) * HD]).astype(bf)
        in_maps.append({
            "xT": xT, "w_slice": w_slice, "wo_slice": wo_slice,
            "pos_sel": pos_sel, "invf": invf, "svec": svec,
            "mask_l": mask_l.astype(bf), "mask_r": mask_r.astype(bf),
            "ones": ones.astype(bf),
        })
    return in_maps


def kernel(positions, hidden_states, w_qkv, w_o):
    nc = _get_nc()
    in_maps = _host_prep(positions, hidden_states, w_qkv, w_o)
    # one retry: transient NRT/device errors (e.g. NRT_EXEC_UNIT_UNRECOVERABLE
    # from a wedged core) were observed to succeed on re-dispatch
    try:
        res = run_bass_kernel_spmd(nc, in_maps, core_ids=list(range(NCORES)))
    except Exception:
        import time
        time.sleep(2.0)
        res = run_bass_kernel_spmd(nc, in_maps, core_ids=list(range(NCORES)))
    yT = np.zeros((HIDDEN, T), dtype=np.float64)
    for c in range(NCORES):
        yT += np.asarray(res.results[c]["yT"], dtype=np.float64)
    return np.ascontiguousarray(yT.T).astype(np.float32)


# revision 13
# speedup vs baseline: 1.0279x; 1.0279x over previous
"""Trainium2 Bass kernel for Ernie4.5-VL attention (mRoPE + GQA causal attention).

Sharding: tensor-parallel over heads across 8 cores. Each core computes
2 q heads + its kv head (replicated per core pair): qkv projection
(q/k feature-major, V token-major directly — no transposes), interleaved
mRoPE (via a host-side even/odd column permutation of the q/k weight
slices so the rotation becomes two contiguous partition halves), causal
attention with unnormalized softmax (denominator via bf16 tile adds +
one all-ones matmul), and the o_proj partial product. Host sums the 8
partial outputs.

All tensors move through SBUF/DRAM as bf16; matmuls are bf16 in / fp32
psum out; psum evacuations round once to bf16. Schedule: a flat
software pipeline where attention chunk g's score->exp->AV steps are
interleaved (emission-order round-robin) with chunk g+1's projection
matmuls and chunk g-1's o_proj — the PE fills exp (ACT) latency with
projection work instead of stalling, which also keeps the PE p-state
ramped.
"""
import numpy as np
import ml_dtypes
from contextlib import ExitStack

import concourse.bacc as bacc
import concourse.tile as tile
from concourse import mybir
from concourse.bass_utils import run_bass_kernel_spmd

HIDDEN = 2048
T = 2048
N_HEADS = 16
N_KV = 4
HD = 128
THETA = 500000.0
NCORES = 8
SCALE = HD ** -0.5

F32 = mybir.dt.float32
BF16 = mybir.dt.bfloat16
I32 = mybir.dt.int32

# within-head column permutation: evens then odds (so interleaved rope pairs
# become two contiguous partition halves in feature-major layout)
PERM = np.concatenate([np.arange(0, HD, 2), np.arange(1, HD, 2)])
# pair index p (0..63): p<44: even->pos row 1 (h), odd->row 2 (w); p>=44: row 0 (t)
ROW_MAP = np.array([(1 if p % 2 == 0 else 2) if p < 44 else 0 for p in range(64)])
INVF = (THETA ** (-(np.arange(64, dtype=np.float64) / 64))).astype(np.float32)

NT = T // 128      # 16 token tiles
NG = T // 512      # 4 token chunks
NH_T = HIDDEN // 128  # 16 hidden tiles


def _build(dbg=False):
    nc = bacc.Bacc("TRN2", target_bir_lowering=False, debug=False)
    d_xT = nc.dram_tensor("xT", [HIDDEN, T], BF16, kind="ExternalInput").ap()
    d_w = nc.dram_tensor("w_slice", [HIDDEN, 512], BF16, kind="ExternalInput").ap()
    d_wo = nc.dram_tensor("wo_slice", [256, HIDDEN], BF16, kind="ExternalInput").ap()
    d_cdup = nc.dram_tensor("cdup", [128, T], BF16, kind="ExternalInput").ap()
    d_sflip = nc.dram_tensor("sflip", [128, T], BF16, kind="ExternalInput").ap()
    d_mL = nc.dram_tensor("mask_l", [128, 128], BF16, kind="ExternalInput").ap()
    d_mR = nc.dram_tensor("mask_r", [128, 4, 512], BF16, kind="ExternalInput").ap()
    d_ones = nc.dram_tensor("ones", [128, 128], BF16, kind="ExternalInput").ap()
    d_yT = nc.dram_tensor("yT", [HIDDEN, T], BF16, kind="ExternalOutput").ap()
    if dbg:
        d_qkv = nc.dram_tensor("dbg_qkv", [128, 3, T], F32, kind="ExternalOutput").ap()
        d_cs = nc.dram_tensor("dbg_cs", [128, 2, T], F32, kind="ExternalOutput").ap()
        d_V = nc.dram_tensor("dbg_V", [128, NT, 128], F32, kind="ExternalOutput").ap()
        d_O = nc.dram_tensor("dbg_O", [128, 2, T], F32, kind="ExternalOutput").ap()

    with tile.TileContext(nc) as tc, ExitStack() as ctx:
        const = ctx.enter_context(tc.tile_pool(name="const", bufs=1))
        big = ctx.enter_context(tc.tile_pool(name="big", bufs=1))

        # resident tiles
        w_sb = const.tile([128, NH_T, 512], BF16)       # qkv weight slice
        wo_sb = const.tile([128, 2, HIDDEN], BF16)      # o_proj rows
        mL_sb = const.tile([128, 128], BF16)            # causal mask, left factor
        mR_sb = const.tile([128, 4, 512], BF16)         # causal mask, right factor
        ones_sb = const.tile([128, 128], BF16)
        qkv_sb = big.tile([128, 3, T], BF16)            # q0|q1|k feature-major (roped)
        V_sb = big.tile([128, NT, 128], BF16)           # V token-major
        O_sb = big.tile([128, 2, T], BF16)              # attention out, feature-major
        cdup = big.tile([128, T], BF16)                 # cos table (dup halves)
        sflip = big.tile([128, T], BF16)                # sin table ([-s; s])

        # PSUM budget (8 banks): q0/q1/k accum 3 + V-direct 1 +
        # shared(scores/o_proj) 3 + AV accum 1.
        xtp = ctx.enter_context(tc.tile_pool(name="xt", bufs=2))
        qkvp = ctx.enter_context(tc.tile_pool(name="qkvp", bufs=3, space="PSUM"))
        vdp = ctx.enter_context(tc.tile_pool(name="vdp", bufs=1, space="PSUM"))
        spp = ctx.enter_context(tc.tile_pool(name="spp", bufs=3, space="PSUM"))
        avp = ctx.enter_context(tc.tile_pool(name="avp", bufs=1, space="PSUM"))
        rp = ctx.enter_context(tc.tile_pool(name="rope", bufs=2))
        ep = ctx.enter_context(tc.tile_pool(name="ep", bufs=6))
        rv = ctx.enter_context(tc.tile_pool(name="rv", bufs=2))
        racc = ctx.enter_context(tc.tile_pool(name="racc", bufs=2))
        yo = ctx.enter_context(tc.tile_pool(name="yo", bufs=2))

        # ---- startup DMAs, ordered so the first projection matmuls unblock
        # as early as possible (cos/sin rope tables are host-computed)
        xt_tiles = {}
        xt_tiles[0] = xtp.tile([128, NH_T, 512], BF16, tag="xt", name="xt_0")
        for q4 in range(4):
            nc.sync.dma_start(
                out=w_sb[:, 4 * q4:4 * (q4 + 1), :],
                in_=d_w[512 * q4:512 * (q4 + 1), :].rearrange(
                    "(a p) c -> p a c", p=128))
            nc.sync.dma_start(
                out=xt_tiles[0][:, 4 * q4:4 * (q4 + 1), :],
                in_=d_xT[512 * q4:512 * (q4 + 1), 0:512].rearrange(
                    "(a p) c -> p a c", p=128))
            if q4 == 1:
                nc.sync.dma_start(out=cdup, in_=d_cdup)
                nc.sync.dma_start(out=sflip, in_=d_sflip)
        nc.sync.dma_start(out=mL_sb, in_=d_mL)
        nc.sync.dma_start(out=mR_sb, in_=d_mR)
        nc.sync.dma_start(out=ones_sb, in_=d_ones)
        nc.sync.dma_start(
            out=wo_sb, in_=d_wo.rearrange("(a p) c -> p a c", p=128))

        # ================= stage emitters =================
        proj_state = {}

        def proj_units(g):
            """Per-hb units of chunk g's projection. First unit issues the
            xt DMA for g (g=0's was issued at startup)."""
            tsl = np.s_[512 * g:512 * (g + 1)]
            units = []

            def alloc():
                if g not in xt_tiles:
                    xt_tiles[g] = xtp.tile([128, NH_T, 512], BF16, tag="xt",
                                           name=f"xt_{g}")
                    nc.sync.dma_start(
                        out=xt_tiles[g],
                        in_=d_xT[:, tsl].rearrange("(a p) c -> p a c", p=128))
                proj_state[g] = {
                    "ps": [qkvp.tile([128, 512], F32, tag="qkvps",
                                     name=f"qkvps_{g}_{i}") for i in range(3)],
                    "vd": vdp.tile([128, 4, 128], F32, tag="vd", name=f"vd_{g}"),
                }

            def mk(hb):
                def emit():
                    if hb == 0:
                        alloc()
                    st = proj_state[g]
                    xt_b = xt_tiles[g]
                    for i in range(3):
                        nc.tensor.matmul(
                            st["ps"][i][:], w_sb[:, hb, 128 * i:128 * (i + 1)],
                            xt_b[:, hb, :],
                            start=(hb == 0), stop=(hb == NH_T - 1))
                    for tt in range(4):
                        nc.tensor.matmul(
                            st["vd"][:, tt, :],
                            xt_b[:, hb, 128 * tt:128 * (tt + 1)],
                            w_sb[:, hb, 384:512],
                            start=(hb == 0), stop=(hb == NH_T - 1))
                return emit

            for hb in range(NH_T):
                units.append(mk(hb))
            return units

        def rope_units(g):
            """V evac (Pool) + mRoPE for k/q0/q1 of chunk g as filler units,
            reading the projection psums (swapped halves via a bf16 scratch
            + SBUF->SBUF DMA), writing bf16 qkv_sb once. k first: it gates
            chunk g's scores."""
            tsl = np.s_[512 * g:512 * (g + 1)]
            xs = rp.tile([128, 3, 512], BF16, tag="xs", name=f"xs{g}")

            def mk_rope(t3):
                def emit():
                    st = proj_state[g]
                    psx = st["ps"][t3]
                    x = qkv_sb[:, t3, tsl]
                    xraw = rp.tile([128, 512], BF16, tag="xraw",
                                   name=f"xr_{g}_{t3}")
                    nc.vector.tensor_copy(xraw[:], psx[:])
                    nc.sync.dma_start(out=xs[0:64, t3, :], in_=xraw[64:128, :])
                    nc.sync.dma_start(out=xs[64:128, t3, :], in_=xraw[0:64, :])
                    t1 = rp.tile([128, 512], F32, tag="t1", name=f"t1_{g}_{t3}")
                    t2 = rp.tile([128, 512], F32, tag="t2", name=f"t2_{g}_{t3}")
                    nc.vector.tensor_mul(t1[:], psx[:], cdup[:, tsl])
                    nc.gpsimd.tensor_mul(t2[:], xs[:, t3, :], sflip[:, tsl])
                    nc.vector.tensor_add(x, t1[:], t2[:])
                return emit

            def mk_vevac():
                def emit():
                    st = proj_state[g]
                    for tt in range(4):
                        nc.gpsimd.tensor_copy(V_sb[:, 4 * g + tt, :],
                                              st["vd"][:, tt, :])
                return emit

            return [mk_rope(2), mk_vevac(), mk_rope(0), mk_rope(1)]

        def attn_steps(g):
            """Flat list of per-j-step emitters for both heads of chunk g.
            Scores run one step ahead of AV; head-0's denominator tail is
            emitted two steps into head 1 so its latency hides behind
            head-1 scores."""
            tsl = np.s_[512 * g:512 * (g + 1)]
            jmax = 4 * g + 4
            state = {}

            def head_alloc(h):
                state[h] = {
                    "po": avp.tile([128, 512], F32, tag="av", name=f"po{g}_{h}"),
                    "ra": racc.tile([128, 512], BF16, tag="ra", name=f"ra{g}_{h}"),
                    "rb": racc.tile([128, 512], BF16, tag="rb", name=f"rb{g}_{h}"),
                    "Es": [None] * jmax,
                }

            def mk_step(h, j):
                def emit():
                    if j == 0:
                        head_alloc(h)
                    st = state[h]
                    qc = qkv_sb[:, h, tsl]
                    m = j - 4 * g
                    ps = spp.tile([128, 512], F32, tag="sp", name=f"s{g}_{h}_{j}")
                    nc.tensor.matmul(ps[:], qkv_sb[:, 2, 128 * j:128 * (j + 1)],
                                     qc, start=True, stop=(m < 0))
                    if m >= 0:
                        # additive causal mask (-1e9 on invalid) via rank-
                        # factored matmul accumulated into the scores psum
                        nc.tensor.matmul(ps[:], mL_sb[:], mR_sb[:, m, :],
                                         start=False, stop=True)
                    E = ep.tile([128, 512], BF16, tag="e", name=f"e{g}_{h}_{j}")
                    st["Es"][j] = E
                    nc.scalar.activation(E[:], ps[:],
                                         mybir.ActivationFunctionType.Exp,
                                         scale=SCALE)
                    # row-sum partials: two bf16 accumulators on DVE
                    if j == 0:
                        nc.vector.tensor_copy(st["ra"][:], E[:])
                    elif j == 1:
                        nc.vector.tensor_copy(st["rb"][:], E[:])
                    elif j % 2 == 0:
                        nc.vector.tensor_add(st["ra"][:], st["ra"][:], E[:])
                    else:
                        nc.vector.tensor_add(st["rb"][:], st["rb"][:], E[:])
                    if j >= 1:
                        nc.tensor.matmul(st["po"][:], V_sb[:, j - 1, :],
                                         st["Es"][j - 1][:],
                                         start=(j == 1), stop=False)
                return emit

            def mk_tail(h):
                def emit():
                    st = state[h]
                    nc.tensor.matmul(st["po"][:], V_sb[:, jmax - 1, :],
                                     st["Es"][jmax - 1][:],
                                     start=(jmax == 1), stop=True)
                    nc.vector.tensor_add(st["ra"][:], st["ra"][:], st["rb"][:])
                    # r broadcast across partitions via one all-ones matmul
                    pr = spp.tile([128, 512], F32, tag="sp", name=f"pr{g}_{h}")
                    nc.tensor.matmul(pr[:], ones_sb[:], st["ra"][:],
                                     start=True, stop=True)
                    rinv = rv.tile([128, 512], F32, tag="rv", name=f"rinv{g}_{h}")
                    nc.vector.reciprocal(rinv[:], pr[:])
                    nc.vector.tensor_mul(O_sb[:, h, tsl], st["po"][:], rinv[:])
                return emit

            steps = [mk_step(0, j) for j in range(jmax)]
            h1 = [mk_step(1, j) for j in range(jmax)]
            steps += h1[:2] + [mk_tail(0)] + h1[2:] + [mk_tail(1)]
            return steps

        def oproj_units(g):
            """o_proj partial chunk: yT[:, tsl] = sum_h wo_h.T @ O_h, with
            psum evacuation rotated over DVE/ACT/Pool and a DMA per 4 tiles."""
            tsl = np.s_[512 * g:512 * (g + 1)]
            ybuf = yo.tile([128, NH_T, 512], BF16, tag="yo", name=f"yb{g}")

            def mk(i):
                def emit():
                    py = spp.tile([128, 512], F32, tag="sp", name=f"y{g}_{i}")
                    for h in range(2):
                        nc.tensor.matmul(py[:], wo_sb[:, h, 128 * i:128 * (i + 1)],
                                         O_sb[:, h, tsl],
                                         start=(h == 0), stop=(h == 1))
                    r = i % 4
                    if r == 1:
                        nc.scalar.copy(ybuf[:, i, :], py[:])
                    elif r == 3:
                        nc.gpsimd.tensor_copy(ybuf[:, i, :], py[:])
                    else:
                        nc.vector.tensor_copy(ybuf[:, i, :], py[:])
                    if i % 4 == 3:
                        nc.sync.dma_start(
                            out=d_yT[512 * (i // 4):512 * (i // 4 + 1),
                                     tsl].rearrange("(a p) c -> p a c", p=128),
                            in_=ybuf[:, i - 3:i + 1, :])
                return emit

            return [mk(i) for i in range(NH_T)]

        def interleave(steps, fillers, lead=0):
            """Emit `lead` fillers up front (PE is in-order: a stalled step
            blocks everything emitted after it, so cover known step-0 latency
            with work emitted before it), then round-robin proportionally."""
            done = 0
            while done < min(lead, len(fillers)):
                fillers[done]()
                done += 1
            for si, s in enumerate(steps):
                s()
                want = max(done, (si + 1) * len(fillers) // len(steps))
                while done < want:
                    fillers[done]()
                    done += 1
            while done < len(fillers):
                fillers[done]()
                done += 1

        # ================= schedule =================
        for u in proj_units(0):
            u()
        for u in rope_units(0):
            u()
        for g in range(NG):
            # filler queue: proj(g+1) and oproj(g-1) alternating (spreads
            # shared-psum pressure), rope(g+1) right after the last proj
            # unit, late oproj units (with their output DMAs) at the end.
            ou = oproj_units(g - 1) if g > 0 else []
            pu = proj_units(g + 1) if g + 1 < NG else []
            ru = rope_units(g + 1) if g + 1 < NG else []
            head, tail = ou[:10], ou[10:]
            mixed = []
            for i in range(max(len(head), len(pu))):
                if i < len(pu):
                    mixed.append(pu[i])
                if i < len(head):
                    mixed.append(head[i])
            fillers = mixed + ru + tail
            interleave(attn_steps(g), fillers, lead=3 if g == 0 else 2)
        for u in oproj_units(NG - 1):
            u()

        if dbg:
            nc.sync.dma_start(out=d_qkv, in_=qkv_sb[:].bitcast(F32))
            nc.sync.dma_start(out=d_cs[:, 0, :], in_=cdup[:].bitcast(F32))
            nc.sync.dma_start(out=d_cs[:, 1, :], in_=sflip[:].bitcast(F32))
            nc.sync.dma_start(out=d_V, in_=V_sb[:].bitcast(F32))
            nc.sync.dma_start(out=d_O, in_=O_sb[:].bitcast(F32))

    nc.compile()
    return nc


_NC_CACHE = None


def _get_nc():
    global _NC_CACHE
    if _NC_CACHE is None:
        _NC_CACHE = _build()
    return _NC_CACHE


def _host_prep(positions, hidden_states, w_qkv, w_o):
    positions = np.asarray(positions, dtype=np.int32)
    hidden_states = np.asarray(hidden_states, dtype=np.float32)
    w_qkv = np.asarray(w_qkv, dtype=np.float32)
    w_o = np.asarray(w_o, dtype=np.float32)

    bf = ml_dtypes.bfloat16
    xT = np.ascontiguousarray(hidden_states.T).astype(bf)
    # rope tables, host-computed: partition p holds rotation pair p%64 with
    # positions from ROW_MAP's t/h/w row. cdup = cos both halves;
    # sflip = [-sin; +sin] (so x*cdup + swap(x)*sflip rotates in place).
    pos_sel = positions[np.concatenate([ROW_MAP, ROW_MAP])].astype(np.float64)
    ang = pos_sel * np.concatenate([INVF, INVF]).astype(np.float64)[:, None]
    cdup = np.cos(ang)
    sflip = np.concatenate([-np.sin(ang[:64]), np.sin(ang[64:])], axis=0)
    # additive causal mask factors: invalid(dk, dq) = [dq - 128m + 1 <= dk]
    #   = sum_p L[p, dk] * Rm[p, dq],  L[p, dk] = [p <= dk],
    #   Rm[p, dq] = [p == max(dq - 128m + 1, 0)]  (scaled by -1e9)
    mask_l = (np.arange(128)[:, None] <= np.arange(128)[None, :]).astype(np.float32)
    mask_r = np.zeros((128, 4, 512), dtype=np.float32)
    for m in range(4):
        c = np.maximum(np.arange(512) - 128 * m + 1, 0)
        valid_rows = c <= 127
        mask_r[c[valid_rows], m, np.arange(512)[valid_rows]] = -1e9
    # sanity: factored mask == boolean causal mask
    dq = np.arange(512)[None, :]
    dk = np.arange(128)[:, None]
    for m in range(4):
        got = mask_l.T @ mask_r[:, m, :]
        want = np.where(dq < dk + 128 * m, -1e9, 0.0)
        assert np.array_equal(got, want), f"mask factorization wrong for m={m}"
    ones = np.ones((128, 128), dtype=np.float32)

    q_size = N_HEADS * HD
    kv_size = N_KV * HD
    in_maps = []
    for c in range(NCORES):
        cols = [w_qkv[:, 2 * c * HD + PERM], w_qkv[:, (2 * c + 1) * HD + PERM]]
        kc = c // 2
        cols.append(w_qkv[:, q_size + kc * HD + PERM])
        cols.append(w_qkv[:, q_size + kv_size + kc * HD:q_size + kv_size + (kc + 1) * HD])
        w_slice = np.ascontiguousarray(np.concatenate(cols, axis=1)).astype(bf)
        wo_slice = np.ascontiguousarray(w_o[2 * c * HD:(2 * c + 2) * HD]).astype(bf)
        in_maps.append({
            "xT": xT, "w_slice": w_slice, "wo_slice": wo_slice,
            "cdup": np.ascontiguousarray(cdup).astype(bf),
            "sflip": np.ascontiguousarray(sflip).astype(bf),
            "mask_l": mask_l.astype(bf), "mask_r": mask_r.astype(bf),
            "ones": ones.astype(bf),
        })
    return in_maps


def kernel(positions, hidden_states, w_qkv, w_o):
    nc = _get_nc()
    in_maps = _host_prep(positions, hidden_states, w_qkv, w_o)
    # one retry: transient NRT/device errors (e.g. NRT_EXEC_UNIT_UNRECOVERABLE
    # from a wedged core) were observed to succeed on re-dispatch
    try:
        res = run_bass_kernel_spmd(nc, in_maps, core_ids=list(range(NCORES)))
    except Exception:
        import time
        time.sleep(2.0)
        res = run_bass_kernel_spmd(nc, in_maps, core_ids=list(range(NCORES)))
    yT = np.zeros((HIDDEN, T), dtype=np.float64)
    for c in range(NCORES):
        yT += np.asarray(res.results[c]["yT"], dtype=np.float64)
    return np.ascontiguousarray(yT.T).astype(np.float32)
